# revision 1
# baseline (speedup 1.0000x reference)
"""Cosine-similarity attention on 8 Trainium2 NeuronCores.

Sharding: 8 cores = (batch, query-half). Each core computes masked cosine
attention for 2048 query rows against all 4096 keys of its batch element.

The device kernel works entirely in the transposed domain ([dim, seq] /
[key, query] layouts) so that the key axis — the softmax reduction axis —
lies on SBUF partitions. Softmax needs no max subtraction (cosine scores
are bounded by 1/temp), so the reduction is a plain sum, computed on the
TensorEngine with a ones-vector matmul. This avoids any on-chip transpose
of the [Sk, Sq]-sized tensors; the host transposes mask in and attn out.
"""

import sys

sys.path.insert(0, "/opt/trn_rl_repo")

import numpy as np

B, SQ, SK, D = 4, 4096, 4096, 128
N_CORES = 8
QS = SQ // (N_CORES // B)  # 2048 query rows per core
QB = 512                   # query block (one PSUM bank wide)
NQB = QS // QB             # 4
NKC = SK // 128            # 32 key chunks

_CACHE = {}


def _build_program():
    from contextlib import ExitStack

    import concourse.bass as bass  # noqa: F401
    import concourse.tile as tile
    from concourse import bacc, mybir

    fp32 = mybir.dt.float32
    AF = mybir.ActivationFunctionType

    nc = bacc.Bacc(None, target_bir_lowering=False, debug=False)
    qT = nc.dram_tensor("qT", [D, QS], fp32, kind="ExternalInput")
    kT = nc.dram_tensor("kT", [D, SK], fp32, kind="ExternalInput")
    v = nc.dram_tensor("v", [SK, D], fp32, kind="ExternalInput")
    maskT = nc.dram_tensor("maskT", [SK, QS], fp32, kind="ExternalInput")
    temp = nc.dram_tensor("temp", [1, 1], fp32, kind="ExternalInput")
    attnT = nc.dram_tensor("attnT", [SK, QS], fp32, kind="ExternalOutput")
    outT = nc.dram_tensor("outT", [D, QS], fp32, kind="ExternalOutput")

    with tile.TileContext(nc) as tc:
        with ExitStack() as ctx:
            persist = ctx.enter_context(tc.tile_pool(name="persist", bufs=1))

            ones_col = persist.tile([1, 128], fp32, tag="ones_col")
            nc.vector.memset(ones_col, 1.0)
            ones_kcol = persist.tile([128, 1], fp32, tag="ones_kcol")
            nc.vector.memset(ones_kcol, 1.0)

            tmp_s = persist.tile([1, 1], fp32, tag="tmp_s")
            nc.sync.dma_start(out=tmp_s, in_=temp[:, :])
            rtemp = persist.tile([1, 1], fp32, tag="rtemp")
            nc.vector.reciprocal(rtemp, tmp_s)

            qnT = persist.tile([128, QS], fp32, tag="qnT")
            nc.sync.dma_start(out=qnT, in_=qT[:, :])
            knT = persist.tile([128, SK], fp32, tag="knT")
            nc.sync.dma_start(out=knT, in_=kT[:, :])
            vt = persist.tile([128, NKC, 128], fp32, tag="vt")
            nc.sync.dma_start(out=vt, in_=v.rearrange("(c p) d -> p c d", p=128))

            # ---- normalize q and k in the transposed layout ----
            # rnorm[s] = 1/sqrt(sum_d x[d,s]^2); column sums via ones-matmul.
            with tc.tile_pool(name="prep", bufs=2) as prep, tc.tile_pool(
                name="prep_ps", bufs=2, space="PSUM"
            ) as prep_ps:
                for name, tile_, width, fold_temp in (
                    ("q", qnT, QS, True),
                    ("k", knT, SK, False),
                ):
                    sq = prep.tile([128, width], fp32, tag="sq")
                    nc.vector.tensor_mul(sq, tile_, tile_)
                    rn = prep.tile([1, width], fp32, tag="rn")
                    for j in range(width // QB):
                        s2 = prep_ps.tile([1, QB], fp32, tag="s2")
                        nc.tensor.matmul(
                            s2, lhsT=ones_kcol, rhs=sq[:, j * QB:(j + 1) * QB],
                            start=True, stop=True,
                        )
                        nc.scalar.activation(
                            rn[:, j * QB:(j + 1) * QB], s2, AF.Sqrt
                        )
                    nc.vector.reciprocal(rn, rn)
                    if fold_temp:
                        nc.vector.tensor_scalar_mul(rn, rn, rtemp)
                    for j in range(width // QB):
                        bc = prep_ps.tile([128, QB], fp32, tag="bc")
                        nc.tensor.matmul(
                            bc, lhsT=ones_col, rhs=rn[:, j * QB:(j + 1) * QB],
                            start=True, stop=True,
                        )
                        sl = slice(j * QB, (j + 1) * QB)
                        nc.vector.tensor_mul(tile_[:, sl], tile_[:, sl], bc)

            # ---- main attention loop ----
            psim = ctx.enter_context(tc.tile_pool(name="psim", bufs=2, space="PSUM"))
            pacc = ctx.enter_context(tc.tile_pool(name="pacc", bufs=2, space="PSUM"))
            psums = ctx.enter_context(tc.tile_pool(name="psums", bufs=2, space="PSUM"))
            prb = ctx.enter_context(tc.tile_pool(name="prb", bufs=1, space="PSUM"))
            pe_pool = ctx.enter_context(tc.tile_pool(name="pe", bufs=3))
            pm_pool = ctx.enter_context(tc.tile_pool(name="pm", bufs=4))
            ptT = ctx.enter_context(tc.tile_pool(name="ptT", bufs=NKC + 2))
            pa_pool = ctx.enter_context(tc.tile_pool(name="pa", bufs=4))
            pmisc = ctx.enter_context(tc.tile_pool(name="pmisc", bufs=3))

            for qb in range(NQB):
                q0 = qb * QB
                acc = pacc.tile([128, QB], fp32, tag="acc")
                sm = psums.tile([1, QB], fp32, tag="sm")
                tts = []
                for kc in range(NKC):
                    k0 = kc * 128
                    sim = psim.tile([128, QB], fp32, tag="sim")
                    nc.tensor.matmul(
                        sim, lhsT=knT[:, k0:k0 + 128], rhs=qnT[:, q0:q0 + QB],
                        start=True, stop=True,
                    )
                    e = pe_pool.tile([128, QB], fp32, tag="e")
                    nc.scalar.activation(e, sim, AF.Exp)
                    m = pm_pool.tile([128, QB], fp32, tag="m")
                    nc.sync.dma_start(out=m, in_=maskT[k0:k0 + 128, q0:q0 + QB])
                    tt = ptT.tile([128, QB], fp32, tag="tT")
                    nc.vector.tensor_mul(tt, e, m)
                    tts.append(tt)
                    nc.tensor.matmul(
                        acc, lhsT=vt[:, kc, :], rhs=tt,
                        start=(kc == 0), stop=(kc == NKC - 1),
                    )
                    nc.tensor.matmul(
                        sm, lhsT=ones_kcol, rhs=tt,
                        start=(kc == 0), stop=(kc == NKC - 1),
                    )
                r = pmisc.tile([1, QB], fp32, tag="r")
                nc.vector.reciprocal(r, sm)
                rb = prb.tile([128, QB], fp32, tag="rb")
                nc.tensor.matmul(rb, lhsT=ones_col, rhs=r, start=True, stop=True)
                rbs = pmisc.tile([128, QB], fp32, tag="rbs")
                nc.scalar.activation(rbs, rb, AF.Copy)
                osb = pmisc.tile([128, QB], fp32, tag="osb")
                nc.vector.tensor_mul(osb, acc, rbs)
                nc.sync.dma_start(out=outT[:, q0:q0 + QB], in_=osb)
                for kc in range(NKC):
                    a = pa_pool.tile([128, QB], fp32, tag="a")
                    nc.vector.tensor_mul(a, tts[kc], rbs)
                    nc.sync.dma_start(
                        out=attnT[kc * 128:(kc + 1) * 128, q0:q0 + QB], in_=a
                    )

    nc.finalize()
    return nc


def get_program():
    if "nc" not in _CACHE:
        _CACHE["nc"] = _build_program()
    return _CACHE["nc"]


def make_in_maps(query, key, value, mask, temp):
    query = np.asarray(query, dtype=np.float32)
    key = np.asarray(key, dtype=np.float32)
    value = np.asarray(value, dtype=np.float32)
    mask = np.asarray(mask)
    temp_arr = np.asarray(temp, dtype=np.float32).reshape(1, 1)
    in_maps = []
    for c in range(N_CORES):
        b, h = divmod(c, N_CORES // B)
        sl = slice(h * QS, (h + 1) * QS)
        in_maps.append({
            "qT": np.ascontiguousarray(query[b, sl, :].T),
            "kT": np.ascontiguousarray(key[b].T),
            "v": np.ascontiguousarray(value[b]),
            "maskT": mask[b, sl, :].T.astype(np.float32),
            "temp": temp_arr,
        })
    return in_maps


def assemble(results):
    out = np.empty((B, SQ, D), dtype=np.float32)
    attn = np.empty((B, SQ, SK), dtype=np.float32)
    for c in range(N_CORES):
        b, h = divmod(c, N_CORES // B)
        sl = slice(h * QS, (h + 1) * QS)
        out[b, sl, :] = results[c]["outT"].T
        attn[b, sl, :] = results[c]["attnT"].T
    return out, attn


def kernel(query, key, value, mask, temp):
    from concourse.bass_utils import run_bass_kernel_spmd

    nc = get_program()
    in_maps = make_in_maps(query, key, value, mask, temp)
    res = run_bass_kernel_spmd(nc, in_maps, list(range(N_CORES)))
    return assemble(res.results)


# revision 4
# speedup vs baseline: 1.6468x; 1.6468x over previous
"""Cosine-similarity attention on 8 Trainium2 NeuronCores.

Sharding: 8 cores = (batch, query-half). Each core computes masked cosine
attention for 2048 query rows against all 4096 keys of its batch element.

The device kernel works entirely in the transposed domain ([dim, seq] /
[key, query] layouts) so that the key axis — the softmax reduction axis —
lies on SBUF partitions. Softmax needs no max subtraction (cosine scores
are bounded by 1/temp), so the reduction is a plain sum, computed on the
TensorEngine with a ones-vector matmul. This avoids any on-chip transpose
of the [Sk, Sq]-sized tensors; the host transposes mask in and attn out.

Matmul operands use float32r (single-pass PE streaming, ~1e-4 rounding)
instead of float32 (which lowers to two half-rate passes). The mask is
shipped as int8 and multiplied directly into the exp'd scores (the DVE
converts on read). Mask loads and attn stores use chunk-packed DRAM
layouts so every DMA touches a fully contiguous block.
"""

import sys

sys.path.insert(0, "/opt/trn_rl_repo")

import numpy as np

B, SQ, SK, D = 4, 4096, 4096, 128
N_CORES = 8
QS = SQ // (N_CORES // B)  # 2048 query rows per core
QB = 512                   # query block (one PSUM bank wide)
NQB = QS // QB             # 4
NKC = SK // 128            # 32 key chunks

_CACHE = {}


def _build_program():
    from contextlib import ExitStack

    import concourse.bass as bass  # noqa: F401
    import concourse.tile as tile
    from concourse import bacc, mybir

    fp32 = mybir.dt.float32
    fp32r = mybir.dt.float32r
    i8 = mybir.dt.int8
    AF = mybir.ActivationFunctionType

    nc = bacc.Bacc(None, target_bir_lowering=False, debug=False)
    qT = nc.dram_tensor("qT", [D, QS], fp32, kind="ExternalInput")
    kT = nc.dram_tensor("kT", [D, SK], fp32, kind="ExternalInput")
    v = nc.dram_tensor("v", [SK, D], fp32r, kind="ExternalInput")
    maskP = nc.dram_tensor("maskP", [NQB, NKC, 128, QB], i8, kind="ExternalInput")
    temp = nc.dram_tensor("temp", [1, 1], fp32, kind="ExternalInput")
    attnP = nc.dram_tensor("attnP", [NQB, NKC, 128, QB], fp32, kind="ExternalOutput")
    outT = nc.dram_tensor("outT", [D, QS], fp32, kind="ExternalOutput")

    with tile.TileContext(nc) as tc:
        with ExitStack() as ctx:
            persist = ctx.enter_context(tc.tile_pool(name="persist", bufs=1))

            ones_col_f = persist.tile([1, 128], fp32, tag="ones_col_f")
            nc.vector.memset(ones_col_f, 1.0)
            ones_col = persist.tile([1, 128], fp32r, tag="ones_col")
            nc.scalar.activation(ones_col, ones_col_f, AF.Copy)
            ones_kcol_f = persist.tile([128, 1], fp32, tag="ones_kcol_f")
            nc.vector.memset(ones_kcol_f, 1.0)
            ones_kcol = persist.tile([128, 1], fp32r, tag="ones_kcol")
            nc.scalar.activation(ones_kcol, ones_kcol_f, AF.Copy)

            tmp_s = persist.tile([1, 1], fp32, tag="tmp_s")
            nc.sync.dma_start(out=tmp_s, in_=temp[:, :])
            rtemp = persist.tile([1, 1], fp32, tag="rtemp")
            nc.vector.reciprocal(rtemp, tmp_s)

            qf = persist.tile([128, QS], fp32, tag="qf")
            nc.sync.dma_start(out=qf, in_=qT[:, :])
            kf = persist.tile([128, SK], fp32, tag="kf")
            nc.sync.dma_start(out=kf, in_=kT[:, :])
            qnT = persist.tile([128, QS], fp32r, tag="qnT")
            knT = persist.tile([128, SK], fp32r, tag="knT")
            vt = persist.tile([128, NKC, 128], fp32r, tag="vt")
            nc.sync.dma_start(out=vt, in_=v.rearrange("(c p) d -> p c d", p=128))

            # ---- normalize q and k in the transposed layout ----
            # rnorm[s] = 1/sqrt(sum_d x[d,s]^2); column sums via ones-matmul.
            # The scaled result is written as float32r (rounded matmul operand).
            with tc.tile_pool(name="prep", bufs=2) as prep, tc.tile_pool(
                name="prep_ps", bufs=2, space="PSUM"
            ) as prep_ps:
                for name, src, dst, width, fold_temp in (
                    ("q", qf, qnT, QS, True),
                    ("k", kf, knT, SK, False),
                ):
                    sq = prep.tile([128, width], fp32r, tag="sq")
                    nc.vector.tensor_mul(sq, src, src)
                    rn = prep.tile([1, width], fp32, tag="rn")
                    for j in range(width // QB):
                        s2 = prep_ps.tile([1, QB], fp32, tag="s2")
                        nc.tensor.matmul(
                            s2, lhsT=ones_kcol,
                            rhs=sq[:, j * QB:(j + 1) * QB].bitcast(fp32r),
                            start=True, stop=True,
                        )
                        nc.scalar.activation(
                            rn[:, j * QB:(j + 1) * QB], s2, AF.Sqrt
                        )
                    rnr = prep.tile([1, width], fp32r, tag="rnr")
                    with nc.allow_low_precision(reason="fp32r matmul operand"):
                        nc.vector.reciprocal(rnr, rn)
                    if fold_temp:
                        nc.vector.tensor_scalar_mul(rnr, rnr, rtemp)
                    for j in range(width // QB):
                        bc = prep_ps.tile([128, QB], fp32, tag="bc")
                        nc.tensor.matmul(
                            bc, lhsT=ones_col, rhs=rnr[:, j * QB:(j + 1) * QB],
                            start=True, stop=True,
                        )
                        sl = slice(j * QB, (j + 1) * QB)
                        nc.vector.tensor_mul(dst[:, sl], src[:, sl], bc)

            # ---- main attention loop ----
            psim = ctx.enter_context(tc.tile_pool(name="psim", bufs=2, space="PSUM"))
            pacc = ctx.enter_context(tc.tile_pool(name="pacc", bufs=2, space="PSUM"))
            psums = ctx.enter_context(tc.tile_pool(name="psums", bufs=2, space="PSUM"))
            prb = ctx.enter_context(tc.tile_pool(name="prb", bufs=2, space="PSUM"))
            pe_pool = ctx.enter_context(tc.tile_pool(name="pe", bufs=3))
            pm_pool = ctx.enter_context(tc.tile_pool(name="pm", bufs=4))
            ptT = ctx.enter_context(tc.tile_pool(name="ptT", bufs=NKC + 2))
            pa_pool = ctx.enter_context(tc.tile_pool(name="pa", bufs=6))
            pmisc = ctx.enter_context(tc.tile_pool(name="pmisc", bufs=3))

            for qb in range(NQB):
                q0 = qb * QB
                acc = pacc.tile([128, QB], fp32, tag="acc")
                sm = psums.tile([1, QB], fp32, tag="sm")
                tts = []
                for kc in range(NKC):
                    k0 = kc * 128
                    sim = psim.tile([128, QB], fp32, tag="sim")
                    nc.tensor.matmul(
                        sim, lhsT=knT[:, k0:k0 + 128], rhs=qnT[:, q0:q0 + QB],
                        start=True, stop=True,
                    )
                    e = pe_pool.tile([128, QB], fp32r, tag="e")
                    nc.scalar.activation(e, sim, AF.Exp)
                    m = pm_pool.tile([128, QB], i8, tag="m")
                    nc.sync.dma_start(out=m, in_=maskP[qb, kc, :, :])
                    tt = ptT.tile([128, QB], fp32r, tag="tT")
                    nc.vector.tensor_mul(tt, e, m)
                    tts.append(tt)
                    nc.tensor.matmul(
                        acc, lhsT=vt[:, kc, :], rhs=tt,
                        start=(kc == 0), stop=(kc == NKC - 1),
                    )
                    nc.tensor.matmul(
                        sm, lhsT=ones_kcol, rhs=tt,
                        start=(kc == 0), stop=(kc == NKC - 1),
                    )
                r = pmisc.tile([1, QB], fp32r, tag="r")
                with nc.allow_low_precision(reason="fp32r matmul operand"):
                    nc.vector.reciprocal(r, sm)
                rb = prb.tile([128, QB], fp32, tag="rb")
                nc.tensor.matmul(rb, lhsT=ones_col, rhs=r, start=True, stop=True)
                rbs = pmisc.tile([128, QB], fp32, tag="rbs")
                nc.scalar.activation(rbs, rb, AF.Copy)
                osb = pmisc.tile([128, QB], fp32, tag="osb")
                nc.vector.tensor_mul(osb, acc, rbs)
                nc.sync.dma_start(out=outT[:, q0:q0 + QB], in_=osb)
                for kc in range(NKC):
                    a = pa_pool.tile([128, QB], fp32, tag="a")
                    eng = nc.gpsimd if (kc % 2) else nc.vector
                    eng.tensor_mul(a, tts[kc], rbs)
                    nc.sync.dma_start(out=attnP[qb, kc, :, :], in_=a)

    nc.finalize()
    return nc


def get_program():
    if "nc" not in _CACHE:
        _CACHE["nc"] = _build_program()
    return _CACHE["nc"]


def make_in_maps(query, key, value, mask, temp):
    query = np.asarray(query, dtype=np.float32)
    key = np.asarray(key, dtype=np.float32)
    value = np.asarray(value, dtype=np.float32)
    mask = np.asarray(mask)
    temp_arr = np.asarray(temp, dtype=np.float32).reshape(1, 1)
    in_maps = []
    for c in range(N_CORES):
        b, h = divmod(c, N_CORES // B)
        sl = slice(h * QS, (h + 1) * QS)
        # maskP[qb, kc, p, j] = mask[b, sl][qb*QB+j, kc*128+p]
        mb = mask[b, sl, :].astype(np.int8)            # [QS, SK]
        mp = np.ascontiguousarray(
            mb.reshape(NQB, QB, NKC, 128).transpose(0, 2, 3, 1)
        )
        in_maps.append({
            "qT": np.ascontiguousarray(query[b, sl, :].T),
            "kT": np.ascontiguousarray(key[b].T),
            "v": np.ascontiguousarray(value[b]),
            "maskP": mp,
            "temp": temp_arr,
        })
    return in_maps


def assemble(results):
    out = np.empty((B, SQ, D), dtype=np.float32)
    attn = np.empty((B, SQ, SK), dtype=np.float32)
    for c in range(N_CORES):
        b, h = divmod(c, N_CORES // B)
        sl = slice(h * QS, (h + 1) * QS)
        out[b, sl, :] = results[c]["outT"].T
        # attnP[qb, kc, p, j] -> attn[qb*QB+j, kc*128+p]
        ap = results[c]["attnP"]
        attn[b, sl, :] = ap.transpose(0, 3, 1, 2).reshape(QS, SK)
    return out, attn


def kernel(query, key, value, mask, temp):
    from concourse.bass_utils import run_bass_kernel_spmd

    nc = get_program()
    in_maps = make_in_maps(query, key, value, mask, temp)
    res = run_bass_kernel_spmd(nc, in_maps, list(range(N_CORES)))
    return assemble(res.results)


# revision 5
# speedup vs baseline: 2.1850x; 1.3268x over previous
"""Cosine-similarity attention on 8 Trainium2 NeuronCores.

Sharding: 8 cores = (batch, query-half). Each core computes masked cosine
attention for 2048 query rows against all 4096 keys of its batch element.

The device kernel works entirely in the transposed domain ([dim, seq] /
[key, query] layouts) so that the key axis — the softmax reduction axis —
lies on SBUF partitions. Softmax needs no max subtraction (cosine scores
are bounded by 1/temp). The device produces the unnormalized masked
scores t = exp(sim)*mask (which is also what the attention output is, up
to the row-sum scale) and out_unnorm = t @ V; the softmax denominator and
the final row scale are applied on the host during the unshard/transpose
pass, which already touches every output element.

Matmul operands use float32r (single-pass PE streaming, ~1e-4 rounding)
instead of float32 (which lowers to two half-rate passes). The mask is
shipped as int8 and multiplied directly into the exp'd scores (the DVE
converts on read). Mask loads and attn stores move 8 key-chunks per DMA
with chunk-group-packed DRAM layouts (16 KiB contiguous per partition
row) so each transfer is a large contiguous burst.
"""

import sys

sys.path.insert(0, "/opt/trn_rl_repo")

import numpy as np

B, SQ, SK, D = 4, 4096, 4096, 128
N_CORES = 8
QS = SQ // (N_CORES // B)  # 2048 query rows per core
QB = 512                   # query block (one PSUM bank wide)
NQB = QS // QB             # 4
NKC = SK // 128            # 32 key chunks
G = 8                      # key chunks per wide group
NG = NKC // G              # 4 groups
GW = G * QB                # 4096 wide-group width

_CACHE = {}


def _build_program():
    from contextlib import ExitStack

    import concourse.bass as bass  # noqa: F401
    import concourse.tile as tile
    from concourse import bacc, mybir

    fp32 = mybir.dt.float32
    fp32r = mybir.dt.float32r
    i8 = mybir.dt.int8
    AF = mybir.ActivationFunctionType

    nc = bacc.Bacc(None, target_bir_lowering=False, debug=False)
    qT = nc.dram_tensor("qT", [D, QS], fp32, kind="ExternalInput")
    kT = nc.dram_tensor("kT", [D, SK], fp32, kind="ExternalInput")
    v = nc.dram_tensor("v", [SK, D], fp32r, kind="ExternalInput")
    maskP = nc.dram_tensor("maskP", [NQB, NG, 128, G, QB], i8, kind="ExternalInput")
    temp = nc.dram_tensor("temp", [1, 1], fp32, kind="ExternalInput")
    attnP = nc.dram_tensor(
        "attnP", [NQB, NG, 128, G, QB], fp32r, kind="ExternalOutput"
    )
    outT = nc.dram_tensor("outT", [D, QS], fp32, kind="ExternalOutput")

    with tile.TileContext(nc) as tc:
        with ExitStack() as ctx:
            persist = ctx.enter_context(tc.tile_pool(name="persist", bufs=1))

            ones_col_f = persist.tile([1, 128], fp32, tag="ones_col_f")
            nc.vector.memset(ones_col_f, 1.0)
            ones_col = persist.tile([1, 128], fp32r, tag="ones_col")
            nc.scalar.activation(ones_col, ones_col_f, AF.Copy)
            ones_kcol_f = persist.tile([128, 1], fp32, tag="ones_kcol_f")
            nc.vector.memset(ones_kcol_f, 1.0)
            ones_kcol = persist.tile([128, 1], fp32r, tag="ones_kcol")
            nc.scalar.activation(ones_kcol, ones_kcol_f, AF.Copy)

            tmp_s = persist.tile([1, 1], fp32, tag="tmp_s")
            nc.sync.dma_start(out=tmp_s, in_=temp[:, :])
            rtemp = persist.tile([1, 1], fp32, tag="rtemp")
            nc.vector.reciprocal(rtemp, tmp_s)

            qnT = persist.tile([128, QS], fp32r, tag="qnT")
            knT = persist.tile([128, SK], fp32r, tag="knT")
            vt = persist.tile([128, NKC, 128], fp32r, tag="vt")
            nc.sync.dma_start(out=vt, in_=v.rearrange("(c p) d -> p c d", p=128))

            # ---- normalize q and k in the transposed layout ----
            # rnorm[s] = 1/sqrt(sum_d x[d,s]^2); column sums via ones-matmul.
            # The scaled result is written as float32r (rounded matmul operand).
            with tc.tile_pool(name="prep", bufs=2) as prep, tc.tile_pool(
                name="prep_ps", bufs=2, space="PSUM"
            ) as prep_ps:
                for name, dram_src, dst, width, fold_temp in (
                    ("q", qT, qnT, QS, True),
                    ("k", kT, knT, SK, False),
                ):
                    src = prep.tile([128, width], fp32, tag="src")
                    nc.sync.dma_start(out=src, in_=dram_src[:, :])
                    sq = prep.tile([128, width], fp32r, tag="sq")
                    nc.vector.tensor_mul(sq, src, src)
                    rn = prep.tile([1, width], fp32, tag="rn")
                    for j in range(width // QB):
                        s2 = prep_ps.tile([1, QB], fp32, tag="s2")
                        nc.tensor.matmul(
                            s2, lhsT=ones_kcol,
                            rhs=sq[:, j * QB:(j + 1) * QB],
                            start=True, stop=True,
                        )
                        nc.scalar.activation(
                            rn[:, j * QB:(j + 1) * QB], s2, AF.Sqrt
                        )
                    rnr = prep.tile([1, width], fp32r, tag="rnr")
                    with nc.allow_low_precision(reason="fp32r matmul operand"):
                        nc.vector.reciprocal(rnr, rn)
                    if fold_temp:
                        nc.vector.tensor_scalar_mul(rnr, rnr, rtemp)
                    for j in range(width // QB):
                        bc = prep_ps.tile([128, QB], fp32, tag="bc")
                        nc.tensor.matmul(
                            bc, lhsT=ones_col, rhs=rnr[:, j * QB:(j + 1) * QB],
                            start=True, stop=True,
                        )
                        sl = slice(j * QB, (j + 1) * QB)
                        nc.vector.tensor_mul(dst[:, sl], src[:, sl], bc)

            # ---- main attention loop ----
            psim = ctx.enter_context(tc.tile_pool(name="psim", bufs=4, space="PSUM"))
            pacc = ctx.enter_context(tc.tile_pool(name="pacc", bufs=2, space="PSUM"))
            pe_pool = ctx.enter_context(tc.tile_pool(name="pe", bufs=2))
            pm_pool = ctx.enter_context(tc.tile_pool(name="pm", bufs=3))
            ptT = ctx.enter_context(tc.tile_pool(name="ptT", bufs=5))
            pmisc = ctx.enter_context(tc.tile_pool(name="pmisc", bufs=3))

            for qb in range(NQB):
                q0 = qb * QB
                acc = pacc.tile([128, QB], fp32, tag="acc")
                for g in range(NG):
                    ew = pe_pool.tile([128, GW], fp32r, tag="e")
                    mw = pm_pool.tile([128, GW], i8, tag="m")
                    nc.sync.dma_start(
                        out=mw,
                        in_=maskP[qb, g].rearrange("p c q -> p (c q)"),
                    )
                    for j in range(G):
                        kc = g * G + j
                        k0 = kc * 128
                        sim = psim.tile([128, QB], fp32, tag="sim")
                        nc.tensor.matmul(
                            sim, lhsT=knT[:, k0:k0 + 128],
                            rhs=qnT[:, q0:q0 + QB],
                            start=True, stop=True,
                        )
                        nc.scalar.activation(
                            ew[:, j * QB:(j + 1) * QB], sim, AF.Exp
                        )
                    tw = ptT.tile([128, GW], fp32r, tag="tT")
                    # split the mask-apply across DVE and GpSimd
                    eng = nc.gpsimd if (g == NG - 1) else nc.vector
                    eng.tensor_mul(tw, ew, mw)
                    for j in range(G):
                        kc = g * G + j
                        nc.tensor.matmul(
                            acc, lhsT=vt[:, kc, :],
                            rhs=tw[:, j * QB:(j + 1) * QB],
                            start=(kc == 0), stop=(kc == NKC - 1),
                        )
                    nc.sync.dma_start(
                        out=attnP[qb, g].rearrange("p c q -> p (c q)"), in_=tw
                    )
                osb = pmisc.tile([128, QB], fp32, tag="osb")
                nc.scalar.activation(osb, acc, AF.Copy)
                nc.sync.dma_start(out=outT[:, q0:q0 + QB], in_=osb)

    nc.finalize()
    return nc


def get_program():
    if "nc" not in _CACHE:
        _CACHE["nc"] = _build_program()
    return _CACHE["nc"]


def make_in_maps(query, key, value, mask, temp):
    query = np.asarray(query, dtype=np.float32)
    key = np.asarray(key, dtype=np.float32)
    value = np.asarray(value, dtype=np.float32)
    mask = np.asarray(mask)
    temp_arr = np.asarray(temp, dtype=np.float32).reshape(1, 1)
    in_maps = []
    for c in range(N_CORES):
        b, h = divmod(c, N_CORES // B)
        sl = slice(h * QS, (h + 1) * QS)
        # maskP[qb, g, p, c, q] = mask[b, sl][qb*QB+q, (g*G+c)*128+p]
        mb = mask[b, sl, :].astype(np.int8)            # [QS, SK]
        mp = np.ascontiguousarray(
            mb.reshape(NQB, QB, NG, G, 128).transpose(0, 2, 4, 3, 1)
        )
        in_maps.append({
            "qT": np.ascontiguousarray(query[b, sl, :].T),
            "kT": np.ascontiguousarray(key[b].T),
            "v": np.ascontiguousarray(value[b]),
            "maskP": mp,
            "temp": temp_arr,
        })
    return in_maps


def assemble(results):
    out = np.empty((B, SQ, D), dtype=np.float32)
    attn = np.empty((B, SQ, SK), dtype=np.float32)
    for c in range(N_CORES):
        b, h = divmod(c, N_CORES // B)
        sl = slice(h * QS, (h + 1) * QS)
        ap = results[c]["attnP"]                       # [NQB, NG, 128, G, QB]
        # softmax denominator: sum over keys = axes (NG, 128, G)
        sums = ap.astype(np.float64).sum(axis=(1, 2, 3))  # [NQB, QB]
        r = (1.0 / sums).astype(np.float32)
        # attn[qb*QB+q, (g*G+c)*128+p] = ap[qb, g, p, c, q] * r[qb, q]
        at = ap.transpose(0, 4, 1, 3, 2).reshape(QS, SK)
        at = at * r.reshape(QS, 1)
        attn[b, sl, :] = at
        rq = r.reshape(QS)
        out[b, sl, :] = results[c]["outT"].T * rq[:, None]
    return out, attn


def kernel(query, key, value, mask, temp):
    from concourse.bass_utils import run_bass_kernel_spmd

    nc = get_program()
    in_maps = make_in_maps(query, key, value, mask, temp)
    res = run_bass_kernel_spmd(nc, in_maps, list(range(N_CORES)))
    return assemble(res.results)


# revision 7
# speedup vs baseline: 2.4245x; 1.1096x over previous
"""Cosine-similarity attention on 8 Trainium2 NeuronCores.

Sharding: 8 cores = (batch, query-half). Each core computes masked cosine
attention for 2048 query rows against all 4096 keys of its batch element.

The main loop works in the transposed domain ([dim, seq] / [key, query]
layouts) so the key axis — the softmax reduction axis — lies on SBUF
partitions. Softmax needs no max subtraction (cosine scores are bounded
by 1/temp). The device produces the unnormalized masked scores
t = exp(sim)*mask and out_unnorm = t @ V; the softmax denominator and
row scale are applied on the host during the unshard/transpose pass,
which already touches every output element.

q/k arrive in natural [seq, dim] layout: row norms are computed with
per-partition reductions (fast [128,1] reciprocals instead of one-lane
row reciprocals), then the normalized chunks are transposed on the PE.
k chunks land in 32 independent tiles so the first attention matmuls
start while later k chunks are still being normalized.

Matmul operands use float32r (single-pass PE streaming, ~1e-4 rounding).
The mask is shipped as int8, loaded through GpSimd's DMA queue (so loads
don't queue behind attn stores on SyncE), and multiplied directly into
the exp'd scores. Mask loads and attn stores move 8 key-chunks per DMA
with group-packed DRAM layouts (16 KiB contiguous per partition row).
"""

import sys

sys.path.insert(0, "/opt/trn_rl_repo")

import numpy as np

B, SQ, SK, D = 4, 4096, 4096, 128
N_CORES = 8
QS = SQ // (N_CORES // B)  # 2048 query rows per core
QB = 512                   # query block (one PSUM bank wide)
NQB = QS // QB             # 4
NKC = SK // 128            # 32 key chunks
NQC = QS // 128            # 16 query chunks
G = 8                      # key chunks per wide group
NG = NKC // G              # 4 groups
GW = G * QB                # 4096 wide-group width

_CACHE = {}


def _build_program():
    from contextlib import ExitStack

    import concourse.bass as bass  # noqa: F401
    import concourse.tile as tile
    from concourse import bacc, mybir
    from concourse.masks import make_identity

    fp32 = mybir.dt.float32
    fp32r = mybir.dt.float32r
    i8 = mybir.dt.int8
    AF = mybir.ActivationFunctionType

    nc = bacc.Bacc(None, target_bir_lowering=False, debug=False)
    q = nc.dram_tensor("q", [QS, D], fp32, kind="ExternalInput")
    k = nc.dram_tensor("k", [SK, D], fp32, kind="ExternalInput")
    v = nc.dram_tensor("v", [SK, D], fp32r, kind="ExternalInput")
    maskP = nc.dram_tensor("maskP", [NQB, NG, 128, G, QB], i8, kind="ExternalInput")
    temp = nc.dram_tensor("temp", [1, 1], fp32, kind="ExternalInput")
    attnP = nc.dram_tensor(
        "attnP", [NQB, NG, 128, G, QB], fp32r, kind="ExternalOutput"
    )
    outT = nc.dram_tensor("outT", [D, QS], fp32, kind="ExternalOutput")

    with tile.TileContext(nc) as tc:
        with ExitStack() as ctx:
            persist = ctx.enter_context(tc.tile_pool(name="persist", bufs=1))

            ident = persist.tile([128, 128], fp32, tag="ident")
            make_identity(nc, ident)

            tmp_s = persist.tile([1, 1], fp32, tag="tmp_s")
            nc.sync.dma_start(out=tmp_s, in_=temp[:, :])
            rtemp1 = persist.tile([1, 1], fp32, tag="rtemp1")
            nc.vector.reciprocal(rtemp1, tmp_s)
            rtemp = persist.tile([128, 1], fp32, tag="rtemp")
            nc.gpsimd.partition_broadcast(rtemp, rtemp1)

            qnT = persist.tile([128, QS], fp32r, tag="qnT")
            vt = persist.tile([128, NKC, 128], fp32r, tag="vt")
            nc.sync.dma_start(out=vt, in_=v.rearrange("(c p) d -> p c d", p=128))

            qn_nat = persist.tile([128, NQC, 128], fp32, tag="qn_nat")
            nc.sync.dma_start(out=qn_nat, in_=q.rearrange("(c p) d -> p c d", p=128))
            kn_nat = persist.tile([128, NKC, 128], fp32, tag="kn_nat")
            nc.sync.dma_start(out=kn_nat, in_=k.rearrange("(c p) d -> p c d", p=128))

            pknT = ctx.enter_context(tc.tile_pool(name="pknT", bufs=NKC))
            prep = ctx.enter_context(tc.tile_pool(name="prep", bufs=3))
            prep_ps = ctx.enter_context(
                tc.tile_pool(name="prep_ps", bufs=2, space="PSUM")
            )

            # ---- normalize one 128-row chunk and transpose it on the PE ----
            def norm_chunk(nat_chunk, dst_slice, is_q):
                sqc = prep.tile([128, 128], fp32, tag="sqc")
                nc.vector.tensor_mul(sqc, nat_chunk, nat_chunk)
                s2 = prep.tile([128, 1], fp32, tag="s2")
                nc.vector.reduce_sum(s2, sqc, axis=mybir.AxisListType.X)
                nrm = prep.tile([128, 1], fp32, tag="nrm")
                nc.scalar.activation(nrm, s2, AF.Sqrt)
                rn = prep.tile([128, 1], fp32, tag="rn")
                nc.vector.reciprocal(rn, nrm)
                sc = prep.tile([128, 128], fp32, tag="sc")
                if is_q:
                    nc.vector.tensor_scalar(
                        sc, nat_chunk, rn, rtemp,
                        op0=mybir.AluOpType.mult, op1=mybir.AluOpType.mult,
                    )
                else:
                    nc.vector.tensor_scalar_mul(sc, nat_chunk, rn)
                tp = prep_ps.tile([128, 128], fp32, tag="tp")
                nc.tensor.transpose(tp, sc, ident)
                nc.scalar.activation(dst_slice, tp, AF.Copy)

            for i in range(NQC):
                norm_chunk(qn_nat[:, i, :], qnT[:, i * 128:(i + 1) * 128], True)
            knT = []
            for c in range(NKC):
                kt = pknT.tile([128, 128], fp32r, tag="knT")
                knT.append(kt)
                norm_chunk(kn_nat[:, c, :], kt, False)

            # ---- main attention loop ----
            psim = ctx.enter_context(tc.tile_pool(name="psim", bufs=4, space="PSUM"))
            pacc = ctx.enter_context(tc.tile_pool(name="pacc", bufs=2, space="PSUM"))
            pe_pool = ctx.enter_context(tc.tile_pool(name="pe", bufs=2))
            pm_pool = ctx.enter_context(tc.tile_pool(name="pm", bufs=3))
            ptT = ctx.enter_context(tc.tile_pool(name="ptT", bufs=3))
            pmisc = ctx.enter_context(tc.tile_pool(name="pmisc", bufs=3))

            for qb in range(NQB):
                q0 = qb * QB
                acc = pacc.tile([128, QB], fp32, tag="acc")
                for g in range(NG):
                    ew = pe_pool.tile([128, GW], fp32r, tag="e")
                    mw = pm_pool.tile([128, GW], i8, tag="m")
                    nc.gpsimd.dma_start(
                        out=mw,
                        in_=maskP[qb, g].rearrange("p c q -> p (c q)"),
                    )
                    for j in range(G):
                        kc = g * G + j
                        sim = psim.tile([128, QB], fp32, tag="sim")
                        nc.tensor.matmul(
                            sim, lhsT=knT[kc],
                            rhs=qnT[:, q0:q0 + QB],
                            start=True, stop=True,
                        )
                        nc.scalar.activation(
                            ew[:, j * QB:(j + 1) * QB], sim, AF.Exp
                        )
                    tw = ptT.tile([128, GW], fp32r, tag="tT")
                    # split the mask-apply across DVE and GpSimd
                    eng = nc.gpsimd if (g == NG - 1) else nc.vector
                    eng.tensor_mul(tw, ew, mw)
                    for j in range(G):
                        kc = g * G + j
                        nc.tensor.matmul(
                            acc, lhsT=vt[:, kc, :],
                            rhs=tw[:, j * QB:(j + 1) * QB],
                            start=(kc == 0), stop=(kc == NKC - 1),
                        )
                    nc.sync.dma_start(
                        out=attnP[qb, g].rearrange("p c q -> p (c q)"), in_=tw
                    )
                osb = pmisc.tile([128, QB], fp32, tag="osb")
                nc.scalar.activation(osb, acc, AF.Copy)
                nc.sync.dma_start(out=outT[:, q0:q0 + QB], in_=osb)

    nc.finalize()
    return nc


def get_program():
    if "nc" not in _CACHE:
        _CACHE["nc"] = _build_program()
    return _CACHE["nc"]


def make_in_maps(query, key, value, mask, temp):
    query = np.asarray(query, dtype=np.float32)
    key = np.asarray(key, dtype=np.float32)
    value = np.asarray(value, dtype=np.float32)
    mask = np.asarray(mask)
    temp_arr = np.asarray(temp, dtype=np.float32).reshape(1, 1)
    in_maps = []
    for c in range(N_CORES):
        b, h = divmod(c, N_CORES // B)
        sl = slice(h * QS, (h + 1) * QS)
        # maskP[qb, g, p, c, q] = mask[b, sl][qb*QB+q, (g*G+c)*128+p]
        mb = mask[b, sl, :].astype(np.int8)            # [QS, SK]
        mp = np.ascontiguousarray(
            mb.reshape(NQB, QB, NG, G, 128).transpose(0, 2, 4, 3, 1)
        )
        in_maps.append({
            "q": np.ascontiguousarray(query[b, sl, :]),
            "k": np.ascontiguousarray(key[b]),
            "v": np.ascontiguousarray(value[b]),
            "maskP": mp,
            "temp": temp_arr,
        })
    return in_maps


def assemble(results):
    out = np.empty((B, SQ, D), dtype=np.float32)
    attn = np.empty((B, SQ, SK), dtype=np.float32)
    for c in range(N_CORES):
        b, h = divmod(c, N_CORES // B)
        sl = slice(h * QS, (h + 1) * QS)
        ap = results[c]["attnP"]                       # [NQB, NG, 128, G, QB]
        # softmax denominator: sum over keys = axes (NG, 128, G)
        sums = ap.astype(np.float64).sum(axis=(1, 2, 3))  # [NQB, QB]
        r = (1.0 / sums).astype(np.float32)
        # attn[qb*QB+q, (g*G+c)*128+p] = ap[qb, g, p, c, q] * r[qb, q]
        at = ap.transpose(0, 4, 1, 3, 2).reshape(QS, SK)
        at = at * r.reshape(QS, 1)
        attn[b, sl, :] = at
        rq = r.reshape(QS)
        out[b, sl, :] = results[c]["outT"].T * rq[:, None]
    return out, attn


def kernel(query, key, value, mask, temp):
    from concourse.bass_utils import run_bass_kernel_spmd

    nc = get_program()
    in_maps = make_in_maps(query, key, value, mask, temp)
    res = run_bass_kernel_spmd(nc, in_maps, list(range(N_CORES)))
    return assemble(res.results)


# revision 8
# speedup vs baseline: 2.8048x; 1.1569x over previous
"""Cosine-similarity attention on 8 Trainium2 NeuronCores.

Sharding: 8 cores = (batch, query-half). Each core computes masked cosine
attention for 2048 query rows against all 4096 keys of its batch element.

The main loop works in the transposed domain ([dim, seq] / [key, query]
layouts) so the key axis — the softmax reduction axis — lies on SBUF
partitions. Softmax needs no max subtraction (cosine scores are bounded
by 1/temp). The device produces the unnormalized masked scores
t = exp(sim)*mask and out_unnorm = t @ V; the softmax denominator and
row scale are applied on the host during the unshard/transpose pass,
which already touches every output element.

q/k arrive in natural [seq, dim] layout: row norms are computed with
per-partition reductions (fast [128,1] reciprocals instead of one-lane
row reciprocals), then the normalized chunks are transposed on the PE.
k chunks land in 32 independent tiles so the first attention matmuls
start while later k chunks are still being normalized.

Matmul operands use float32r (single-pass PE streaming, ~1e-4 rounding).
The mask is shipped as int8, loaded through GpSimd's DMA queue (so loads
don't queue behind attn stores on SyncE), and multiplied directly into
the exp'd scores. Mask loads and attn stores move 8 key-chunks per DMA
with group-packed DRAM layouts (16 KiB contiguous per partition row).
"""

import sys

sys.path.insert(0, "/opt/trn_rl_repo")

import numpy as np

B, SQ, SK, D = 4, 4096, 4096, 128
N_CORES = 8
QS = SQ // (N_CORES // B)  # 2048 query rows per core
QB = 512                   # query block (one PSUM bank wide)
NQB = QS // QB             # 4
NKC = SK // 128            # 32 key chunks
NQC = QS // 128            # 16 query chunks
G = 8                      # key chunks per wide group
NG = NKC // G              # 4 groups
GW = G * QB                # 4096 wide-group width

_CACHE = {}


def _build_program():
    from contextlib import ExitStack

    import concourse.bass as bass  # noqa: F401
    import concourse.tile as tile
    from concourse import bacc, mybir
    from concourse.masks import make_identity

    fp32 = mybir.dt.float32
    fp32r = mybir.dt.float32r
    i8 = mybir.dt.int8
    AF = mybir.ActivationFunctionType

    nc = bacc.Bacc(None, target_bir_lowering=False, debug=False)
    q = nc.dram_tensor("q", [QS, D], fp32, kind="ExternalInput")
    k = nc.dram_tensor("k", [SK, D], fp32, kind="ExternalInput")
    v = nc.dram_tensor("v", [SK, D], fp32r, kind="ExternalInput")
    maskP = nc.dram_tensor("maskP", [NQB, NG, 128, G, QB], i8, kind="ExternalInput")
    temp = nc.dram_tensor("temp", [1, 1], fp32, kind="ExternalInput")
    attnP = nc.dram_tensor(
        "attnP", [NQB, NG, 128, G, QB], fp32r, kind="ExternalOutput"
    )
    outT = nc.dram_tensor("outT", [D, QS], fp32, kind="ExternalOutput")

    with tile.TileContext(nc) as tc:
        with ExitStack() as ctx:
            persist = ctx.enter_context(tc.tile_pool(name="persist", bufs=1))

            ident = persist.tile([128, 128], fp32, tag="ident")
            make_identity(nc, ident)

            tmp_s = persist.tile([1, 1], fp32, tag="tmp_s")
            nc.sync.dma_start(out=tmp_s, in_=temp[:, :])
            rtemp1 = persist.tile([1, 1], fp32, tag="rtemp1")
            nc.vector.reciprocal(rtemp1, tmp_s)
            rtemp = persist.tile([128, 1], fp32, tag="rtemp")
            nc.gpsimd.partition_broadcast(rtemp, rtemp1)

            qnT = persist.tile([128, QS], fp32r, tag="qnT")
            vt = persist.tile([128, NKC, 128], fp32r, tag="vt")
            nc.sync.dma_start(out=vt, in_=v.rearrange("(c p) d -> p c d", p=128))

            qn_nat = persist.tile([128, NQC, 128], fp32, tag="qn_nat")
            nc.sync.dma_start(out=qn_nat, in_=q.rearrange("(c p) d -> p c d", p=128))
            kn_nat = persist.tile([128, NKC, 128], fp32, tag="kn_nat")
            nc.sync.dma_start(out=kn_nat, in_=k.rearrange("(c p) d -> p c d", p=128))

            pknT = ctx.enter_context(tc.tile_pool(name="pknT", bufs=NKC))
            prep = ctx.enter_context(tc.tile_pool(name="prep", bufs=3))
            prep_ps = ctx.enter_context(
                tc.tile_pool(name="prep_ps", bufs=2, space="PSUM")
            )

            # Row sums-of-squares for all chunks, batched so the scalar
            # engine runs exactly two Sqrt ops (no activation-table thrash
            # against the main loop's Exp).
            s2q = persist.tile([128, NQC], fp32, tag="s2q")
            s2k = persist.tile([128, NKC], fp32, tag="s2k")
            for i in range(NQC):
                sqc = prep.tile([128, 128], fp32, tag="sqc")
                nc.vector.tensor_mul(sqc, qn_nat[:, i, :], qn_nat[:, i, :])
                nc.vector.reduce_sum(
                    s2q[:, i:i + 1], sqc, axis=mybir.AxisListType.X
                )
            for c in range(NKC):
                sqc = prep.tile([128, 128], fp32, tag="sqc")
                nc.vector.tensor_mul(sqc, kn_nat[:, c, :], kn_nat[:, c, :])
                nc.vector.reduce_sum(
                    s2k[:, c:c + 1], sqc, axis=mybir.AxisListType.X
                )
            nrmq = persist.tile([128, NQC], fp32, tag="nrmq")
            nc.scalar.activation(nrmq, s2q, AF.Sqrt)
            rnq = persist.tile([128, NQC], fp32, tag="rnq")
            nc.vector.reciprocal(rnq, nrmq)
            nrmk = persist.tile([128, NKC], fp32, tag="nrmk")
            nc.scalar.activation(nrmk, s2k, AF.Sqrt)
            rnk = persist.tile([128, NKC], fp32, tag="rnk")
            nc.vector.reciprocal(rnk, nrmk)

            # q: scale by 1/(||q||*temp) in natural layout, transpose on PE.
            for i in range(NQC):
                sc = prep.tile([128, 128], fp32, tag="sc")
                nc.vector.tensor_scalar(
                    sc, qn_nat[:, i, :], rnq[:, i:i + 1], rtemp,
                    op0=mybir.AluOpType.mult, op1=mybir.AluOpType.mult,
                )
                tp = prep_ps.tile([128, 128], fp32, tag="tp")
                nc.tensor.transpose(tp, sc, ident)
                with nc.allow_low_precision(reason="fp32r matmul operand"):
                    nc.vector.tensor_copy(qnT[:, i * 128:(i + 1) * 128], tp)
            # k: transpose RAW rows; the 1/||k|| scale is folded into the
            # per-partition `scale` operand of the main loop's Exp.
            knT = []
            for c in range(NKC):
                kt = pknT.tile([128, 128], fp32r, tag="knT")
                knT.append(kt)
                tp = prep_ps.tile([128, 128], fp32, tag="tp")
                nc.tensor.transpose(tp, kn_nat[:, c, :], ident)
                with nc.allow_low_precision(reason="fp32r matmul operand"):
                    nc.vector.tensor_copy(kt, tp)

            # ---- main attention loop ----
            psim = ctx.enter_context(tc.tile_pool(name="psim", bufs=4, space="PSUM"))
            pacc = ctx.enter_context(tc.tile_pool(name="pacc", bufs=2, space="PSUM"))
            pe_pool = ctx.enter_context(tc.tile_pool(name="pe", bufs=2))
            pm_pool = ctx.enter_context(tc.tile_pool(name="pm", bufs=3))
            ptT = ctx.enter_context(tc.tile_pool(name="ptT", bufs=3))
            pmisc = ctx.enter_context(tc.tile_pool(name="pmisc", bufs=3))

            for qb in range(NQB):
                q0 = qb * QB
                acc = pacc.tile([128, QB], fp32, tag="acc")
                for g in range(NG):
                    ew = pe_pool.tile([128, GW], fp32r, tag="e")
                    mw = pm_pool.tile([128, GW], i8, tag="m")
                    nc.gpsimd.dma_start(
                        out=mw,
                        in_=maskP[qb, g].rearrange("p c q -> p (c q)"),
                    )
                    for j in range(G):
                        kc = g * G + j
                        sim = psim.tile([128, QB], fp32, tag="sim")
                        nc.tensor.matmul(
                            sim, lhsT=knT[kc],
                            rhs=qnT[:, q0:q0 + QB],
                            start=True, stop=True,
                        )
                        nc.scalar.activation(
                            ew[:, j * QB:(j + 1) * QB], sim, AF.Exp,
                            scale=rnk[:, kc:kc + 1],
                        )
                    tw = ptT.tile([128, GW], fp32r, tag="tT")
                    # split the mask-apply across DVE and GpSimd
                    eng = nc.gpsimd if (g == NG - 1) else nc.vector
                    eng.tensor_mul(tw, ew, mw)
                    for j in range(G):
                        kc = g * G + j
                        nc.tensor.matmul(
                            acc, lhsT=vt[:, kc, :],
                            rhs=tw[:, j * QB:(j + 1) * QB],
                            start=(kc == 0), stop=(kc == NKC - 1),
                        )
                    nc.sync.dma_start(
                        out=attnP[qb, g].rearrange("p c q -> p (c q)"), in_=tw
                    )
                osb = pmisc.tile([128, QB], fp32, tag="osb")
                nc.vector.tensor_copy(osb, acc)
                nc.sync.dma_start(out=outT[:, q0:q0 + QB], in_=osb)

    nc.finalize()
    return nc


def get_program():
    if "nc" not in _CACHE:
        _CACHE["nc"] = _build_program()
    return _CACHE["nc"]


def make_in_maps(query, key, value, mask, temp):
    query = np.asarray(query, dtype=np.float32)
    key = np.asarray(key, dtype=np.float32)
    value = np.asarray(value, dtype=np.float32)
    mask = np.asarray(mask)
    temp_arr = np.asarray(temp, dtype=np.float32).reshape(1, 1)
    in_maps = []
    for c in range(N_CORES):
        b, h = divmod(c, N_CORES // B)
        sl = slice(h * QS, (h + 1) * QS)
        # maskP[qb, g, p, c, q] = mask[b, sl][qb*QB+q, (g*G+c)*128+p]
        mb = mask[b, sl, :].astype(np.int8)            # [QS, SK]
        mp = np.ascontiguousarray(
            mb.reshape(NQB, QB, NG, G, 128).transpose(0, 2, 4, 3, 1)
        )
        in_maps.append({
            "q": np.ascontiguousarray(query[b, sl, :]),
            "k": np.ascontiguousarray(key[b]),
            "v": np.ascontiguousarray(value[b]),
            "maskP": mp,
            "temp": temp_arr,
        })
    return in_maps


def assemble(results):
    out = np.empty((B, SQ, D), dtype=np.float32)
    attn = np.empty((B, SQ, SK), dtype=np.float32)
    for c in range(N_CORES):
        b, h = divmod(c, N_CORES // B)
        sl = slice(h * QS, (h + 1) * QS)
        ap = results[c]["attnP"]                       # [NQB, NG, 128, G, QB]
        # softmax denominator: sum over keys = axes (NG, 128, G)
        sums = ap.astype(np.float64).sum(axis=(1, 2, 3))  # [NQB, QB]
        r = (1.0 / sums).astype(np.float32)
        # attn[qb*QB+q, (g*G+c)*128+p] = ap[qb, g, p, c, q] * r[qb, q]
        at = ap.transpose(0, 4, 1, 3, 2).reshape(QS, SK)
        at = at * r.reshape(QS, 1)
        attn[b, sl, :] = at
        rq = r.reshape(QS)
        out[b, sl, :] = results[c]["outT"].T * rq[:, None]
    return out, attn


def kernel(query, key, value, mask, temp):
    from concourse.bass_utils import run_bass_kernel_spmd

    nc = get_program()
    in_maps = make_in_maps(query, key, value, mask, temp)
    res = run_bass_kernel_spmd(nc, in_maps, list(range(N_CORES)))
    return assemble(res.results)
